# revision 1
# baseline (speedup 1.0000x reference)
"""Distributed Bass kernel for nn_AttentionCircuit (B=2,S=2048,D=2048,RANK=512,H=16).

Sharding: 8 cores = 2 batches x 4 head-groups (4 heads / 512 D-cols each).
All matmuls in float32r (TF32-like, 4x faster than fp32 on TensorE).

Per-core dataflow (everything laid out so matmul contraction lands on the
partition axis with no on-device transposes; host pre-transposes x / gates):
  A2: t_v^T  = v_read  @ x^T   (stream x^T), gate with g_V^T -> Vg^T
  B2: V      = Vg^T.T @ v_write_hg                       (natural [s,d'])
  A1: t_qk^T = qk_read @ x^T   (stream x^T), gate -> Qg^T, Kg^T
  B1: Q^T/K^T = qk_write_hg.T @ {Q,K}g^T                 (transposed [d',s])
  C:  per si-chunk, per head: scores^T = K^T.T Q^T -> exp (no max-sub; scores
      are small) -> causal mask (block-skip + 4 static masks) -> rowsum via
      ones-matmul -> PV matmul -> normalize w/ 1/(0.81*rowsum)  -> AO^T chunk
      -> AllGather(group of 4) -> D: out_cols = AO_full^T.T @ W_O[:,cols],
      overlapped chunk-wise with C.
"""
import sys
import numpy as np

sys.path.insert(0, '/opt/trn_rl_repo')

import concourse.bass as bass  # noqa: E402
from concourse import bacc  # noqa: E402
import concourse.mybir as mybir  # noqa: E402
import concourse.tile as tile  # noqa: E402
from concourse.bass_utils import run_bass_kernel_spmd  # noqa: E402

B, S, D = 2, 2048, 2048
RANK = 512
NH = 16
HG = 4              # head-groups == cores per batch
DHG = D // HG       # 512 cols per head-group (4 heads)
DH = D // NH        # 128 head dim
P = 128
DB = D // P         # 16 d-blocks
RB = RANK // P      # 4 rank-blocks
SB = S // P         # 16 s-blocks
NT = S // 512       # 4 si tiles of 512
SC = 256            # phase-A s-chunk width
NSC = S // SC       # 8

F32 = mybir.dt.float32
F32R = mybir.dt.float32r
AF = mybir.ActivationFunctionType
ALU = mybir.AluOpType

EXP_SCALE = 1.0 / float(np.sqrt(DH))
INV_KEEP2 = 1.0 / (0.9 * 0.9)
RGROUPS = [[0, 1, 2, 3], [4, 5, 6, 7]]

_CACHE = {}


def _r(ap):
    """[ (o p), f ] DRAM tensor -> [p, o, f] partition-tiled view."""
    return ap.rearrange("(o p) f -> p o f", p=P)


def _build():
    nc = bacc.Bacc("TRN2", target_bir_lowering=False, debug=False,
                   enable_asserts=False, num_devices=8)
    xT = nc.dram_tensor("xT", [D, S], F32, kind="ExternalInput").ap()
    gqT = nc.dram_tensor("gqT", [RANK, S], F32, kind="ExternalInput").ap()
    gkT = nc.dram_tensor("gkT", [RANK, S], F32, kind="ExternalInput").ap()
    gvT = nc.dram_tensor("gvT", [RANK, S], F32, kind="ExternalInput").ap()
    qk_readT = nc.dram_tensor("qk_readT", [D, RANK], F32, kind="ExternalInput").ap()
    v_readT = nc.dram_tensor("v_readT", [D, RANK], F32, kind="ExternalInput").ap()
    qk_w = nc.dram_tensor("qk_write_hg", [RANK, DHG], F32, kind="ExternalInput").ap()
    v_w = nc.dram_tensor("v_write_hg", [RANK, DHG], F32, kind="ExternalInput").ap()
    wo = nc.dram_tensor("wo_cols", [D, DHG], F32, kind="ExternalInput").ap()
    out = nc.dram_tensor("out", [S, DHG], F32, kind="ExternalOutput").ap()

    with tile.TileContext(nc) as tc:
        _body(tc, xT, gqT, gkT, gvT, qk_readT, v_readT, qk_w, v_w, wo, out)
    nc.compile()
    return nc


def _body(tc, xT, gqT, gkT, gvT, qk_readT, v_readT, qk_w, v_w, wo, out):
    nc = tc.nc
    import contextlib
    ctx = contextlib.ExitStack()
    with ctx:
        # ---- long-lived activation tensors (allocated in phase order)
        pool_qk = ctx.enter_context(tc.tile_pool(name="qk", bufs=1))
        QT_sb = pool_qk.tile([P, HG, S], F32R)          # Q^T [d', s]
        KT_sb = pool_qk.tile([P, HG, S], F32R)

        # ========== A1+B1 fused per s-chunk: Q^T, K^T ==========
        with (
            tc.tile_pool(name="qkread", bufs=1) as pool_qr,
            tc.tile_pool(name="qkw", bufs=1) as pool_qw,
            tc.tile_pool(name="ax1", bufs=2) as pool_x,
            tc.tile_pool(name="ag1", bufs=2) as pool_g,
            tc.tile_pool(name="gch1", bufs=2) as pool_gch,
            tc.tile_pool(name="psA1", bufs=4, space="PSUM") as psA,
            tc.tile_pool(name="psB1", bufs=2, space="PSUM") as psB,
        ):
            qr_sb = pool_qr.tile([P, DB, RANK], F32R)
            nc.sync.dma_start(qr_sb[:], _r(qk_readT).bitcast(F32R))
            qw_sb = pool_qw.tile([P, RB, DHG], F32R)
            nc.sync.dma_start(qw_sb[:], _r(qk_w).bitcast(F32R))
            for sc_i in range(NSC):
                sl = slice(sc_i * SC, (sc_i + 1) * SC)
                xt = pool_x.tile([P, DB, SC], F32R, tag="xt")
                nc.sync.dma_start(xt[:], _r(xT)[:, :, sl].bitcast(F32R))
                gq = pool_g.tile([P, RB, SC], F32, tag="gq")
                nc.sync.dma_start(gq[:], _r(gqT)[:, :, sl])
                gk = pool_g.tile([P, RB, SC], F32, tag="gk")
                nc.sync.dma_start(gk[:], _r(gkT)[:, :, sl])
                qg_ch = pool_gch.tile([P, RB, SC], F32R, tag="qg")
                kg_ch = pool_gch.tile([P, RB, SC], F32R, tag="kg")
                for rb in range(RB):
                    ps = psA.tile([P, SC], F32, tag="tA")
                    for db in range(DB):
                        nc.tensor.matmul(ps[:], qr_sb[:, db, rb * P:(rb + 1) * P],
                                         xt[:, db, :], start=(db == 0),
                                         stop=(db == DB - 1))
                    nc.vector.tensor_tensor(qg_ch[:, rb, :], ps[:], gq[:, rb, :],
                                            ALU.mult)
                    nc.vector.tensor_tensor(kg_ch[:, rb, :], ps[:], gk[:, rb, :],
                                            ALU.mult)
                for db in range(HG):
                    dsl = slice(db * P, (db + 1) * P)
                    psq = psB.tile([P, SC], F32, tag="qB")
                    for rb in range(RB):
                        nc.tensor.matmul(psq[:], qw_sb[:, rb, dsl], qg_ch[:, rb, :],
                                         start=(rb == 0), stop=(rb == RB - 1))
                    nc.scalar.activation(QT_sb[:, db, sl], psq[:], AF.Copy)
                    psk = psB.tile([P, SC], F32, tag="kB")
                    for rb in range(RB):
                        nc.tensor.matmul(psk[:], qw_sb[:, rb, dsl], kg_ch[:, rb, :],
                                         start=(rb == 0), stop=(rb == RB - 1))
                    nc.scalar.activation(KT_sb[:, db, sl], psk[:], AF.Copy)

        # ========== A2+B2 fused per s-chunk: V ==========
        pool_v = ctx.enter_context(tc.tile_pool(name="v", bufs=1))
        V_sb = pool_v.tile([P, SB, DHG], F32R)          # V natural [s, d']
        with (
            tc.tile_pool(name="vread", bufs=1) as pool_vr,
            tc.tile_pool(name="vw", bufs=1) as pool_vw,
            tc.tile_pool(name="ax2", bufs=2) as pool_x,
            tc.tile_pool(name="ag2", bufs=2) as pool_g,
            tc.tile_pool(name="gch2", bufs=2) as pool_gch,
            tc.tile_pool(name="psA2", bufs=4, space="PSUM") as psA,
            tc.tile_pool(name="psB2", bufs=4, space="PSUM") as psB,
        ):
            vr_sb = pool_vr.tile([P, DB, RANK], F32R)
            nc.sync.dma_start(vr_sb[:], _r(v_readT).bitcast(F32R))
            vw_sb = pool_vw.tile([P, RB, DHG], F32R)
            nc.sync.dma_start(vw_sb[:], _r(v_w).bitcast(F32R))
            for sc_i in range(NSC):
                sl = slice(sc_i * SC, (sc_i + 1) * SC)
                xt = pool_x.tile([P, DB, SC], F32R, tag="xt")
                nc.sync.dma_start(xt[:], _r(xT)[:, :, sl].bitcast(F32R))
                gv = pool_g.tile([P, RB, SC], F32, tag="gv")
                nc.sync.dma_start(gv[:], _r(gvT)[:, :, sl])
                vg_ch = pool_gch.tile([P, RB, SC], F32R, tag="vg")
                for rb in range(RB):
                    ps = psA.tile([P, SC], F32, tag="tA")
                    for db in range(DB):
                        nc.tensor.matmul(ps[:], vr_sb[:, db, rb * P:(rb + 1) * P],
                                         xt[:, db, :], start=(db == 0),
                                         stop=(db == DB - 1))
                    nc.vector.tensor_tensor(vg_ch[:, rb, :], ps[:], gv[:, rb, :],
                                            ALU.mult)
                for sj in range(SC // P):
                    s_blk = sc_i * (SC // P) + sj
                    psv = psB.tile([P, DHG], F32, tag="vB")
                    for rb in range(RB):
                        nc.tensor.matmul(psv[:], vg_ch[:, rb, sj * P:(sj + 1) * P],
                                         vw_sb[:, rb, :], start=(rb == 0),
                                         stop=(rb == RB - 1))
                    nc.scalar.activation(V_sb[:, s_blk, :], psv[:], AF.Copy)

        # ========== C + D: attention, AllGather, W_O ==========
        with (
            tc.tile_pool(name="csmall", bufs=1) as pool_c1,
            tc.tile_pool(name="exp", bufs=4) as pool_exp,
            tc.tile_pool(name="rep", bufs=2) as pool_rep,
            tc.tile_pool(name="recip", bufs=2) as pool_recip,
            tc.tile_pool(name="ao", bufs=2) as pool_ao,
            tc.tile_pool(name="wo", bufs=1) as pool_wo,
            tc.tile_pool(name="aof", bufs=3) as pool_aof,
            tc.tile_pool(name="dramb", bufs=5, space="DRAM") as pool_dram,
            tc.tile_pool(name="psC", bufs=2, space="PSUM") as psC,
            tc.tile_pool(name="psC1", bufs=2, space="PSUM") as psC1,
            tc.tile_pool(name="psD", bufs=1, space="PSUM") as psD,
        ):
            # constants
            masks = pool_c1.tile([P, HG, 512], F32)
            nc.vector.memset(masks[:], 1.0)
            for o in range(HG):
                nc.gpsimd.affine_select(
                    out=masks[:, o, :], in_=masks[:, o, :],
                    compare_op=ALU.is_ge, fill=0.0, base=-P * o,
                    pattern=[[1, 512]], channel_multiplier=-1)
            ones_f = pool_c1.tile([P, 1], F32)
            nc.vector.memset(ones_f[:], 0.9 * 0.9)
            ones_r = pool_c1.tile([P, 1], F32R)
            nc.vector.tensor_copy(ones_r[:], ones_f[:])
            onecol = pool_c1.tile([1, P], F32)
            nc.vector.memset(onecol[:], 1.0)
            wo_sb = pool_wo.tile([P, DB, DHG], F32R)
            nc.sync.dma_start(wo_sb[:], _r(wo).bitcast(F32R))

            def head_tail(ao, h, pv, rs):
                """normalize head h: recip(rowsum) -> DMA-replicate -> scale pv."""
                recip = pool_recip.tile([1, 512], F32, tag="recip")
                nc.vector.reciprocal(recip[:], rs[:])
                rep_ps = psD.tile([P, 512], F32, tag="rep")
                nc.tensor.matmul(rep_ps[:], onecol[:], recip[:],
                                 start=True, stop=True)
                rep_sb = pool_rep.tile([P, 512], F32, tag="repsb")
                nc.scalar.activation(rep_sb[:], rep_ps[:], AF.Copy)
                nc.vector.tensor_tensor(ao[:, h, :], pv[:], rep_sb[:], ALU.mult)

            ag_outs = []
            for t in range(NT):
                tsl = slice(t * 512, (t + 1) * 512)
                ao = pool_ao.tile([P, HG, 512], F32R, tag="ao")
                nsj = 4 * (t + 1)
                prev = None
                for h in range(HG):
                    pv = psC.tile([P, 512], F32, tag="pv")
                    rs = psC1.tile([1, 512], F32, tag="rs")
                    for j in range(nsj):
                        jsl = slice(j * P, (j + 1) * P)
                        sc = psC.tile([P, 512], F32, tag="sc")
                        nc.tensor.matmul(sc[:], KT_sb[:, h, jsl],
                                         QT_sb[:, h, tsl], start=True, stop=True)
                        et = pool_exp.tile([P, 512], F32R, tag="et")
                        nc.scalar.activation(et[:], sc[:], AF.Exp,
                                             scale=EXP_SCALE)
                        o = j - 4 * t
                        if o >= 0:
                            nc.vector.tensor_tensor(
                                et[:], et[:], masks[:, o, :].bitcast(F32R),
                                ALU.mult)
                        nc.tensor.matmul(rs[:], ones_r[:], et[:],
                                         start=(j == 0), stop=(j == nsj - 1))
                        nc.tensor.matmul(pv[:], V_sb[:, j, h * P:(h + 1) * P],
                                         et[:], start=(j == 0),
                                         stop=(j == nsj - 1))
                        if j == 0 and prev is not None:
                            head_tail(ao, *prev)   # overlap prior head's tail
                            prev = None
                    prev = (h, pv, rs)
                head_tail(ao, *prev)
                # AllGather this si-chunk across the 4-core group
                bin_t = pool_dram.tile([DHG, 512], F32, tag="bin")
                bout_t = pool_dram.tile([D, 512], F32, tag="bout")
                nc.sync.dma_start(
                    bin_t.rearrange("(h p) s -> p h s", p=P), ao[:].bitcast(F32))
                nc.gpsimd.collective_compute(
                    "AllGather", ALU.bypass, ins=[bin_t[:].opt()],
                    outs=[bout_t[:].opt()], replica_groups=RGROUPS)
                ag_outs.append(bout_t)
            # D: all output chunks emitted after the last AG so D(0..2) fill
            # the final AllGather's latency on PE.
            for t in range(NT):
                bout_t = ag_outs[t]
                for si in range(4):
                    aof = pool_aof.tile([P, DB, P], F32R, tag="aof")
                    nc.sync.dma_start(
                        aof[:],
                        _r(bout_t)[:, :, si * P:(si + 1) * P].bitcast(F32R))
                    ps = psD.tile([P, DHG], F32, tag="d")
                    for dbk in range(DB):
                        nc.tensor.matmul(ps[:], aof[:, dbk, :], wo_sb[:, dbk, :],
                                         start=(dbk == 0), stop=(dbk == DB - 1))
                    o_sb = pool_rep.tile([P, DHG], F32, tag="osb")
                    nc.scalar.activation(o_sb[:], ps[:], AF.Copy)
                    row0 = (t * 4 + si) * P
                    nc.sync.dma_start(out[row0:row0 + P, :], o_sb[:])


def _get_nc():
    if 'nc' not in _CACHE:
        _CACHE['nc'] = _build()
    return _CACHE['nc']


def kernel(**inputs):
    x = np.asarray(inputs["x"], np.float32)
    g_Q = np.asarray(inputs["g_Q"], np.float32)
    g_K = np.asarray(inputs["g_K"], np.float32)
    g_V = np.asarray(inputs["g_V"], np.float32)
    qk_read = np.asarray(inputs["qk_read"], np.float32)
    qk_write = np.asarray(inputs["qk_write"], np.float32)
    v_read = np.asarray(inputs["v_read"], np.float32)
    v_write = np.asarray(inputs["v_write"], np.float32)
    W_O = np.asarray(inputs["W_O"], np.float32)

    nc = _get_nc()
    qk_readT = np.ascontiguousarray(qk_read.T)
    v_readT = np.ascontiguousarray(v_read.T)
    in_maps = []
    for c in range(8):
        b, hg = divmod(c, 4)
        cs = slice(hg * DHG, (hg + 1) * DHG)
        in_maps.append({
            "xT": np.ascontiguousarray(x[b].T),
            "gqT": np.ascontiguousarray(g_Q[b].T),
            "gkT": np.ascontiguousarray(g_K[b].T),
            "gvT": np.ascontiguousarray(g_V[b].T),
            "qk_readT": qk_readT,
            "v_readT": v_readT,
            "qk_write_hg": np.ascontiguousarray(qk_write[:, cs]),
            "v_write_hg": np.ascontiguousarray(v_write[:, cs]),
            "wo_cols": np.ascontiguousarray(W_O[:, cs]),
        })
    res = run_bass_kernel_spmd(nc, in_maps, core_ids=list(range(8)))
    _CACHE['last_results'] = res
    out = np.empty((B, S, D), np.float32)
    for c in range(8):
        b, hg = divmod(c, 4)
        out[b, :, hg * DHG:(hg + 1) * DHG] = res.results[c]["out"]
    return out



# revision 2
# speedup vs baseline: 1.2088x; 1.2088x over previous
"""Distributed Bass kernel for nn_AttentionCircuit (B=2,S=2048,D=2048,RANK=512,H=16).

Sharding: 8 cores = 2 batches x 4 head-groups (4 heads / 512 D-cols each).
All matmuls in bf16 (1 cycle/row on PE, half the DMA/SBUF of fp32).

Per-core dataflow (contraction always on the partition axis, no on-device
transposes; host pre-transposes x / gates):
  AB (fused, one pass over x^T, s-chunks of 512):
     t_qk^T = qk_read @ x^T ; gate -> Qg^T, Kg^T   (bf16)
     t_v^T  = v_read  @ x^T ; gate -> Vg^T
     Q^T/K^T = qk_write_hg.T @ {Q,K}g^T   (transposed [d',s], bf16 SBUF)
     V       = Vg^T.T @ v_write_hg        (natural [s,d'], bf16 SBUF)
  C: per 512-query chunk t, per head: scores^T = K^T.T Q^T -> exp (bf16, no
     max-sub; logits are tiny) -> causal mask (block-skip + diag masks) ->
     replicated rowsum via ones128-matmul -> PV matmul -> ao = pv * recip
     -> AllGather(group of 4, bf16) overlapped with next chunks
  D: out_cols = AO_full^T.T @ W_O[:,cols], chunks interleaved after AG(t)
     completes so the readback DMA spreads across phase C.
"""
import sys
import numpy as np
import ml_dtypes

sys.path.insert(0, '/opt/trn_rl_repo')

import concourse.bass as bass  # noqa: E402
from concourse import bacc  # noqa: E402
import concourse.mybir as mybir  # noqa: E402
import concourse.tile as tile  # noqa: E402
from concourse.bass_utils import run_bass_kernel_spmd  # noqa: E402

B, S, D = 2, 2048, 2048
RANK = 512
NH = 16
HG = 4              # head-groups == cores per batch
DHG = D // HG       # 512 cols per head-group (4 heads)
DH = D // NH        # 128 head dim
P = 128
DB = D // P         # 16 d-blocks
RB = RANK // P      # 4 rank-blocks
SB = S // P         # 16 s-blocks
NT = S // 512       # 4 si tiles of 512
SC = 512            # fused-A s-chunk width
NSC = S // SC       # 4

F32 = mybir.dt.float32
BF16 = mybir.dt.bfloat16
AF = mybir.ActivationFunctionType
ALU = mybir.AluOpType

EXP_SCALE = 1.0 / float(np.sqrt(DH))
KEEP2_BF = float(np.float32(ml_dtypes.bfloat16(0.81)))  # ones128 constant as hw sees it
RGROUPS = [[0, 1, 2, 3], [4, 5, 6, 7]]

_CACHE = {}


def _r(ap):
    """[ (o p), f ] DRAM tensor -> [p, o, f] partition-tiled view."""
    return ap.rearrange("(o p) f -> p o f", p=P)


def _build():
    nc = bacc.Bacc("TRN2", target_bir_lowering=False, debug=False,
                   enable_asserts=False, num_devices=8)
    xT = nc.dram_tensor("xT", [D, S], BF16, kind="ExternalInput").ap()
    gqT = nc.dram_tensor("gqT", [RANK, S], BF16, kind="ExternalInput").ap()
    gkT = nc.dram_tensor("gkT", [RANK, S], BF16, kind="ExternalInput").ap()
    gvT = nc.dram_tensor("gvT", [RANK, S], BF16, kind="ExternalInput").ap()
    qk_readT = nc.dram_tensor("qk_readT", [D, RANK], BF16, kind="ExternalInput").ap()
    v_readT = nc.dram_tensor("v_readT", [D, RANK], BF16, kind="ExternalInput").ap()
    qk_w = nc.dram_tensor("qk_write_hg", [RANK, DHG], BF16, kind="ExternalInput").ap()
    v_w = nc.dram_tensor("v_write_hg", [RANK, DHG], BF16, kind="ExternalInput").ap()
    wo = nc.dram_tensor("wo_cols", [D, DHG], BF16, kind="ExternalInput").ap()
    out = nc.dram_tensor("out", [S, DHG], F32, kind="ExternalOutput").ap()

    with tile.TileContext(nc) as tc:
        _body(tc, xT, gqT, gkT, gvT, qk_readT, v_readT, qk_w, v_w, wo, out)
    nc.compile()
    return nc


def _body(tc, xT, gqT, gkT, gvT, qk_readT, v_readT, qk_w, v_w, wo, out):
    nc = tc.nc
    import contextlib
    ctx = contextlib.ExitStack()
    with ctx:
        # ---- long-lived activation tensors
        pool_qk = ctx.enter_context(tc.tile_pool(name="qk", bufs=1))
        QT_sb = pool_qk.tile([P, HG, S], BF16)          # Q^T [d', s]
        KT_sb = pool_qk.tile([P, HG, S], BF16)
        V_sb = pool_qk.tile([P, SB, DHG], BF16)         # V natural [s, d']

        # ========== fused A+B per s-chunk: Q^T, K^T, V ==========
        with (
            tc.tile_pool(name="rd", bufs=1) as pool_rd,
            tc.tile_pool(name="wr", bufs=1) as pool_wr,
            tc.tile_pool(name="ax", bufs=2) as pool_x,
            tc.tile_pool(name="ag", bufs=2) as pool_g,
            tc.tile_pool(name="gch", bufs=2) as pool_gch,
            tc.tile_pool(name="psA", bufs=4, space="PSUM") as psA,
            tc.tile_pool(name="psB", bufs=1, space="PSUM") as psB,
        ):
            qr_sb = pool_rd.tile([P, DB, RANK], BF16)
            vr_sb = pool_rd.tile([P, DB, RANK], BF16)
            # per-block DMAs so the first matmul starts as soon as its
            # slice lands (subtile deps)
            for db in range(DB):
                nc.sync.dma_start(qr_sb[:, db, :], _r(qk_readT)[:, db, :])
            for db in range(DB):
                nc.sync.dma_start(vr_sb[:, db, :], _r(v_readT)[:, db, :])
            qw_sb = pool_wr.tile([P, RB, DHG], BF16)
            nc.sync.dma_start(qw_sb[:], _r(qk_w))
            vw_sb = pool_wr.tile([P, RB, DHG], BF16)
            nc.sync.dma_start(vw_sb[:], _r(v_w))
            for sc_i in range(NSC):
                sl = slice(sc_i * SC, (sc_i + 1) * SC)
                xt = pool_x.tile([P, DB, SC], BF16, tag="xt")
                for db in range(DB):
                    nc.sync.dma_start(xt[:, db, :], _r(xT)[:, db, sl])
                gq = pool_g.tile([P, RB, SC], BF16, tag="gq")
                nc.sync.dma_start(gq[:], _r(gqT)[:, :, sl])
                gk = pool_g.tile([P, RB, SC], BF16, tag="gk")
                nc.sync.dma_start(gk[:], _r(gkT)[:, :, sl])
                gv = pool_g.tile([P, RB, SC], BF16, tag="gv")
                nc.sync.dma_start(gv[:], _r(gvT)[:, :, sl])
                qg = pool_gch.tile([P, RB, SC], BF16, tag="qg")
                kg = pool_gch.tile([P, RB, SC], BF16, tag="kg")
                vg = pool_gch.tile([P, RB, SC], BF16, tag="vg")
                for rb in range(RB):
                    ps = psA.tile([P, SC], F32, tag="tA")
                    for db in range(DB):
                        nc.tensor.matmul(ps[:], qr_sb[:, db, rb * P:(rb + 1) * P],
                                         xt[:, db, :], start=(db == 0),
                                         stop=(db == DB - 1))
                    nc.vector.tensor_tensor(qg[:, rb, :], ps[:], gq[:, rb, :],
                                            ALU.mult)
                    nc.vector.tensor_tensor(kg[:, rb, :], ps[:], gk[:, rb, :],
                                            ALU.mult)
                for rb in range(RB):
                    ps = psA.tile([P, SC], F32, tag="tA")
                    for db in range(DB):
                        nc.tensor.matmul(ps[:], vr_sb[:, db, rb * P:(rb + 1) * P],
                                         xt[:, db, :], start=(db == 0),
                                         stop=(db == DB - 1))
                    nc.vector.tensor_tensor(vg[:, rb, :], ps[:], gv[:, rb, :],
                                            ALU.mult)
                # B1: Q^T / K^T [d', s] per 128-col block of DHG
                for dbk in range(HG):
                    dsl = slice(dbk * P, (dbk + 1) * P)
                    psq = psB.tile([P, SC], F32, tag="qB")
                    for rb in range(RB):
                        nc.tensor.matmul(psq[:], qw_sb[:, rb, dsl], qg[:, rb, :],
                                         start=(rb == 0), stop=(rb == RB - 1))
                    nc.scalar.activation(QT_sb[:, dbk, sl], psq[:], AF.Copy)
                    psk = psB.tile([P, SC], F32, tag="kB")
                    for rb in range(RB):
                        nc.tensor.matmul(psk[:], qw_sb[:, rb, dsl], kg[:, rb, :],
                                         start=(rb == 0), stop=(rb == RB - 1))
                    nc.scalar.activation(KT_sb[:, dbk, sl], psk[:], AF.Copy)
                # B2: V natural [s, d']
                for sj in range(SC // P):
                    s_blk = sc_i * (SC // P) + sj
                    psv = psB.tile([P, DHG], F32, tag="vB", bufs=2)
                    for rb in range(RB):
                        nc.tensor.matmul(psv[:], vg[:, rb, sj * P:(sj + 1) * P],
                                         vw_sb[:, rb, :], start=(rb == 0),
                                         stop=(rb == RB - 1))
                    nc.scalar.activation(V_sb[:, s_blk, :], psv[:], AF.Copy)

        # ========== C + D: attention, AllGather, W_O ==========
        with (
            tc.tile_pool(name="csmall", bufs=1) as pool_c1,
            tc.tile_pool(name="exp", bufs=4) as pool_exp,
            tc.tile_pool(name="recip", bufs=2) as pool_recip,
            tc.tile_pool(name="ao", bufs=2) as pool_ao,
            tc.tile_pool(name="wo", bufs=1) as pool_wo,
            tc.tile_pool(name="aof", bufs=3) as pool_aof,
            tc.tile_pool(name="osb", bufs=2) as pool_osb,
            tc.tile_pool(name="dramb", bufs=4, space="DRAM") as pool_dram,
            tc.tile_pool(name="psS", bufs=2, space="PSUM") as psS,
            tc.tile_pool(name="psPV", bufs=2, space="PSUM") as psPV,
            tc.tile_pool(name="psRS", bufs=2, space="PSUM") as psRS,
            tc.tile_pool(name="psD", bufs=2, space="PSUM") as psD,
        ):
            # constants
            masks_f = pool_c1.tile([P, HG, 512], F32)
            nc.vector.memset(masks_f[:], 1.0)
            for o in range(HG):
                nc.gpsimd.affine_select(
                    out=masks_f[:, o, :], in_=masks_f[:, o, :],
                    compare_op=ALU.is_ge, fill=0.0, base=-P * o,
                    pattern=[[1, 512]], channel_multiplier=-1)
            masks = pool_c1.tile([P, HG, 512], BF16)
            nc.vector.tensor_copy(masks[:], masks_f[:])
            ones128 = pool_c1.tile([P, P], BF16)
            nc.vector.memset(ones128[:], 0.81)  # folds 1/KEEP^2; residual fixed in wo
            wo_sb = pool_wo.tile([P, DB, DHG], BF16)
            for db in range(DB):
                nc.sync.dma_start(wo_sb[:, db, :], _r(wo)[:, db, :])

            def head_tail(ao, h, pv, rsum):
                """ao[:,h,:] = pv / rsum (rowsum already replicated)."""
                recip = pool_recip.tile([P, 512], F32, tag="recip")
                nc.vector.reciprocal(recip[:], rsum[:])
                nc.vector.tensor_tensor(ao[:, h, :], pv[:], recip[:], ALU.mult)

            ag_outs = []

            def emit_D(t):
                bout_t = ag_outs[t]
                for si in range(4):
                    aof = pool_aof.tile([P, DB, P], BF16, tag="aof")
                    nc.sync.dma_start(
                        aof[:], _r(bout_t)[:, :, si * P:(si + 1) * P])
                    ps = psD.tile([P, DHG], F32, tag="d")
                    for dbk in range(DB):
                        nc.tensor.matmul(ps[:], aof[:, dbk, :], wo_sb[:, dbk, :],
                                         start=(dbk == 0), stop=(dbk == DB - 1))
                    o_sb = pool_osb.tile([P, DHG], F32, tag="osb")
                    nc.scalar.activation(o_sb[:], ps[:], AF.Copy)
                    row0 = (t * 4 + si) * P
                    nc.sync.dma_start(out[row0:row0 + P, :], o_sb[:])

            for t in range(NT):
                tsl = slice(t * 512, (t + 1) * 512)
                ao = pool_ao.tile([P, HG, 512], BF16, tag="ao")
                nsj = 4 * (t + 1)
                prev = None
                for h in range(HG):
                    pv = psPV.tile([P, 512], F32, tag="pv")
                    rsum = psRS.tile([P, 512], F32, tag="rs")
                    for j in range(nsj):
                        jsl = slice(j * P, (j + 1) * P)
                        sc = psS.tile([P, 512], F32, tag="sc")
                        nc.tensor.matmul(sc[:], KT_sb[:, h, jsl],
                                         QT_sb[:, h, tsl], start=True, stop=True)
                        et = pool_exp.tile([P, 512], BF16, tag="et")
                        nc.scalar.activation(et[:], sc[:], AF.Exp,
                                             scale=EXP_SCALE)
                        o = j - 4 * t
                        if o >= 0:
                            nc.vector.tensor_tensor(et[:], et[:], masks[:, o, :],
                                                    ALU.mult)
                        nc.tensor.matmul(rsum[:], ones128[:], et[:],
                                         start=(j == 0), stop=(j == nsj - 1))
                        nc.tensor.matmul(pv[:], V_sb[:, j, h * P:(h + 1) * P],
                                         et[:], start=(j == 0),
                                         stop=(j == nsj - 1))
                        if j == 0 and prev is not None:
                            head_tail(ao, *prev)   # overlap prior head's tail
                            prev = None
                    prev = (h, pv, rsum)
                head_tail(ao, *prev)
                # AllGather this si-chunk across the 4-core group (bf16)
                bin_t = pool_dram.tile([DHG, 512], BF16, tag="bin")
                bout_t = pool_dram.tile([D, 512], BF16, tag="bout")
                nc.sync.dma_start(
                    bin_t.rearrange("(h p) s -> p h s", p=P), ao[:])
                nc.gpsimd.collective_compute(
                    "AllGather", ALU.bypass, ins=[bin_t[:].opt()],
                    outs=[bout_t[:].opt()], replica_groups=RGROUPS)
                ag_outs.append(bout_t)
                if t >= 2:
                    emit_D(t - 2)  # AG(t-2) is long done; spreads readback DMA
            emit_D(2)
            emit_D(3)


def _get_nc():
    if 'nc' not in _CACHE:
        _CACHE['nc'] = _build()
    return _CACHE['nc']


def _bf(a):
    return np.ascontiguousarray(a).astype(ml_dtypes.bfloat16)


def kernel(**inputs):
    x = np.asarray(inputs["x"], np.float32)
    g_Q = np.asarray(inputs["g_Q"], np.float32)
    g_K = np.asarray(inputs["g_K"], np.float32)
    g_V = np.asarray(inputs["g_V"], np.float32)
    qk_read = np.asarray(inputs["qk_read"], np.float32)
    qk_write = np.asarray(inputs["qk_write"], np.float32)
    v_read = np.asarray(inputs["v_read"], np.float32)
    v_write = np.asarray(inputs["v_write"], np.float32)
    W_O = np.asarray(inputs["W_O"], np.float32)

    nc = _get_nc()
    qk_readT = _bf(qk_read.T)
    v_readT = _bf(v_read.T)
    # the in-kernel rowsum scale is bf16(0.81); fold the exact residual
    # (and the intended 1/0.81 dropout scaling) into W_O host-side
    wo_fix = W_O * (KEEP2_BF / 0.81)
    in_maps = []
    for c in range(8):
        b, hg = divmod(c, 4)
        cs = slice(hg * DHG, (hg + 1) * DHG)
        in_maps.append({
            "xT": _bf(x[b].T),
            "gqT": _bf(g_Q[b].T),
            "gkT": _bf(g_K[b].T),
            "gvT": _bf(g_V[b].T),
            "qk_readT": qk_readT,
            "v_readT": v_readT,
            "qk_write_hg": _bf(qk_write[:, cs]),
            "v_write_hg": _bf(v_write[:, cs]),
            "wo_cols": _bf(wo_fix[:, cs]),
        })
    res = run_bass_kernel_spmd(nc, in_maps, core_ids=list(range(8)))
    _CACHE['last_results'] = res
    out = np.empty((B, S, D), np.float32)
    for c in range(8):
        b, hg = divmod(c, 4)
        out[b, :, hg * DHG:(hg + 1) * DHG] = res.results[c]["out"]
    return out


# revision 6
# speedup vs baseline: 1.2739x; 1.0538x over previous
"""Distributed Bass kernel for nn_AttentionCircuit (B=2,S=2048,D=2048,RANK=512,H=16).

Sharding: 8 cores = 2 batches x 4 head-groups (4 heads / 512 D-cols each).
All matmuls in bf16 (1 cycle/row on PE, half the DMA/SBUF of fp32).

Per-core dataflow (contraction always on the partition axis, no on-device
transposes; host pre-transposes x / gates):
  AB (fused, one pass over x^T, s-chunks of 512):
     t_qk^T = qk_read @ x^T ; gate -> Qg^T, Kg^T   (bf16)
     t_v^T  = v_read  @ x^T ; gate -> Vg^T
     Q^T/K^T = qk_write_hg.T @ {Q,K}g^T   (transposed [d',s], bf16 SBUF)
     V       = Vg^T.T @ v_write_hg        (natural [s,d'], bf16 SBUF)
  C: per 512-query chunk t, per head: scores^T = K^T.T Q^T -> exp (bf16, no
     max-sub; logits are tiny) -> causal mask (block-skip + diag masks) ->
     replicated rowsum via ones128-matmul -> PV matmul -> ao = pv * recip
     -> AllGather(group of 4, bf16) overlapped with next chunks
  D: out_cols = AO_full^T.T @ W_O[:,cols], chunks interleaved after AG(t)
     completes so the readback DMA spreads across phase C.
"""
import sys
import numpy as np
import ml_dtypes

sys.path.insert(0, '/opt/trn_rl_repo')

import concourse.bass as bass  # noqa: E402
from concourse import bacc  # noqa: E402
import concourse.mybir as mybir  # noqa: E402
import concourse.tile as tile  # noqa: E402
from concourse.bass_utils import run_bass_kernel_spmd  # noqa: E402

B, S, D = 2, 2048, 2048
RANK = 512
NH = 16
HG = 4              # head-groups == cores per batch
DHG = D // HG       # 512 cols per head-group (4 heads)
DH = D // NH        # 128 head dim
P = 128
DB = D // P         # 16 d-blocks
RB = RANK // P      # 4 rank-blocks
SB = S // P         # 16 s-blocks
NT = S // 512       # 4 si tiles of 512
SC = 512            # fused-A s-chunk width
NSC = S // SC       # 4

F32 = mybir.dt.float32
BF16 = mybir.dt.bfloat16
AF = mybir.ActivationFunctionType
ALU = mybir.AluOpType

EXP_SCALE = 1.0 / float(np.sqrt(DH))
KEEP2_BF = float(np.float32(ml_dtypes.bfloat16(0.81)))  # ones128 constant as hw sees it
RGROUPS = [[0, 1, 2, 3], [4, 5, 6, 7]]

_CACHE = {}


def _r(ap):
    """[ (o p), f ] DRAM tensor -> [p, o, f] partition-tiled view."""
    return ap.rearrange("(o p) f -> p o f", p=P)


def _build():
    nc = bacc.Bacc("TRN2", target_bir_lowering=False, debug=False,
                   enable_asserts=False, num_devices=8)
    xT = nc.dram_tensor("xT", [D, S], BF16, kind="ExternalInput").ap()
    gqT = nc.dram_tensor("gqT", [RANK, S], BF16, kind="ExternalInput").ap()
    gkT = nc.dram_tensor("gkT", [RANK, S], BF16, kind="ExternalInput").ap()
    gvT = nc.dram_tensor("gvT", [RANK, S], BF16, kind="ExternalInput").ap()
    qk_readT = nc.dram_tensor("qk_readT", [D, RANK], BF16, kind="ExternalInput").ap()
    v_readT = nc.dram_tensor("v_readT", [D, RANK], BF16, kind="ExternalInput").ap()
    qk_w = nc.dram_tensor("qk_write_hg", [RANK, DHG], BF16, kind="ExternalInput").ap()
    v_w = nc.dram_tensor("v_write_hg", [RANK, DHG], BF16, kind="ExternalInput").ap()
    wo = nc.dram_tensor("wo_cols", [D, DHG], BF16, kind="ExternalInput").ap()
    out = nc.dram_tensor("out", [S, DHG], F32, kind="ExternalOutput").ap()

    with tile.TileContext(nc) as tc:
        _body(tc, xT, gqT, gkT, gvT, qk_readT, v_readT, qk_w, v_w, wo, out)
    nc.compile()
    return nc


def _body(tc, xT, gqT, gkT, gvT, qk_readT, v_readT, qk_w, v_w, wo, out):
    nc = tc.nc
    import contextlib
    ctx = contextlib.ExitStack()
    with ctx:
        # ---- long-lived activation tensors
        pool_qk = ctx.enter_context(tc.tile_pool(name="qk", bufs=1))
        QT_sb = pool_qk.tile([P, HG, S], BF16)          # Q^T [d', s]
        KT_sb = pool_qk.tile([P, HG, S], BF16)
        V_sb = pool_qk.tile([P, SB, DHG], BF16)         # V natural [s, d']

        # ========== fused A+B per s-chunk: Q^T, K^T, V ==========
        with (
            tc.tile_pool(name="rd", bufs=1) as pool_rd,
            tc.tile_pool(name="wr", bufs=1) as pool_wr,
            tc.tile_pool(name="ax", bufs=2) as pool_x,
            tc.tile_pool(name="ag", bufs=2) as pool_g,
            tc.tile_pool(name="gch", bufs=2) as pool_gch,
            tc.tile_pool(name="psA", bufs=4, space="PSUM") as psA,
            tc.tile_pool(name="psB", bufs=1, space="PSUM") as psB,
        ):
            qr_sb = pool_rd.tile([P, DB, RANK], BF16)
            vr_sb = pool_rd.tile([P, DB, RANK], BF16)
            qw_sb = pool_wr.tile([P, RB, DHG], BF16)
            vw_sb = pool_wr.tile([P, RB, DHG], BF16)
            xt0 = pool_x.tile([P, DB, SC], BF16, tag="xt")
            # interleave the read-matrix / x-chunk-0 block DMAs so the first
            # A matmuls start after ~2 blocks land (subtile deps)
            for db in range(DB):
                nc.sync.dma_start(qr_sb[:, db, :], _r(qk_readT)[:, db, :])
                nc.sync.dma_start(xt0[:, db, :], _r(xT)[:, db, 0:SC])
            for db in range(DB):
                nc.sync.dma_start(vr_sb[:, db, :], _r(v_readT)[:, db, :])
            nc.sync.dma_start(qw_sb[:], _r(qk_w))
            nc.sync.dma_start(vw_sb[:], _r(v_w))
            for sc_i in range(NSC):
                sl = slice(sc_i * SC, (sc_i + 1) * SC)
                if sc_i == 0:
                    xt = xt0
                else:
                    xt = pool_x.tile([P, DB, SC], BF16, tag="xt")
                    nc.sync.dma_start(xt[:], _r(xT)[:, :, sl])
                gq = pool_g.tile([P, RB, SC], BF16, tag="gq")
                nc.sync.dma_start(gq[:], _r(gqT)[:, :, sl])
                gk = pool_g.tile([P, RB, SC], BF16, tag="gk")
                nc.sync.dma_start(gk[:], _r(gkT)[:, :, sl])
                gv = pool_g.tile([P, RB, SC], BF16, tag="gv")
                nc.sync.dma_start(gv[:], _r(gvT)[:, :, sl])
                qg = pool_gch.tile([P, RB, SC], BF16, tag="qg")
                kg = pool_gch.tile([P, RB, SC], BF16, tag="kg")
                vg = pool_gch.tile([P, RB, SC], BF16, tag="vg")
                for rb in range(RB):
                    ps = psA.tile([P, SC], F32, tag="tA")
                    for db in range(DB):
                        nc.tensor.matmul(ps[:], qr_sb[:, db, rb * P:(rb + 1) * P],
                                         xt[:, db, :], start=(db == 0),
                                         stop=(db == DB - 1))
                    nc.vector.tensor_tensor(qg[:, rb, :], ps[:], gq[:, rb, :],
                                            ALU.mult)
                    nc.vector.tensor_tensor(kg[:, rb, :], ps[:], gk[:, rb, :],
                                            ALU.mult)
                for rb in range(RB):
                    ps = psA.tile([P, SC], F32, tag="tA")
                    for db in range(DB):
                        nc.tensor.matmul(ps[:], vr_sb[:, db, rb * P:(rb + 1) * P],
                                         xt[:, db, :], start=(db == 0),
                                         stop=(db == DB - 1))
                    nc.vector.tensor_tensor(vg[:, rb, :], ps[:], gv[:, rb, :],
                                            ALU.mult)
                # B1: Q^T / K^T [d', s] per 128-col block of DHG
                for dbk in range(HG):
                    dsl = slice(dbk * P, (dbk + 1) * P)
                    psq = psB.tile([P, SC], F32, tag="qB")
                    for rb in range(RB):
                        nc.tensor.matmul(psq[:], qw_sb[:, rb, dsl], qg[:, rb, :],
                                         start=(rb == 0), stop=(rb == RB - 1))
                    nc.scalar.activation(QT_sb[:, dbk, sl], psq[:], AF.Copy)
                    psk = psB.tile([P, SC], F32, tag="kB")
                    for rb in range(RB):
                        nc.tensor.matmul(psk[:], qw_sb[:, rb, dsl], kg[:, rb, :],
                                         start=(rb == 0), stop=(rb == RB - 1))
                    nc.scalar.activation(KT_sb[:, dbk, sl], psk[:], AF.Copy)
                # B2: V natural [s, d']
                for sj in range(SC // P):
                    s_blk = sc_i * (SC // P) + sj
                    psv = psB.tile([P, DHG], F32, tag="vB", bufs=2)
                    for rb in range(RB):
                        nc.tensor.matmul(psv[:], vg[:, rb, sj * P:(sj + 1) * P],
                                         vw_sb[:, rb, :], start=(rb == 0),
                                         stop=(rb == RB - 1))
                    nc.scalar.activation(V_sb[:, s_blk, :], psv[:], AF.Copy)

        # ========== C + D: attention, AllGather, W_O ==========
        with (
            tc.tile_pool(name="csmall", bufs=1) as pool_c1,
            tc.tile_pool(name="exp", bufs=4) as pool_exp,
            tc.tile_pool(name="recip", bufs=2) as pool_recip,
            tc.tile_pool(name="ao", bufs=2) as pool_ao,
            tc.tile_pool(name="wo", bufs=1) as pool_wo,
            tc.tile_pool(name="aof", bufs=2) as pool_aof,
            tc.tile_pool(name="osb", bufs=2) as pool_osb,
            tc.tile_pool(name="dramb", bufs=4, space="DRAM") as pool_dram,
            tc.tile_pool(name="psS", bufs=2, space="PSUM") as psS,
            tc.tile_pool(name="psPV", bufs=2, space="PSUM") as psPV,
            tc.tile_pool(name="psRS", bufs=2, space="PSUM") as psRS,
            tc.tile_pool(name="psD", bufs=2, space="PSUM") as psD,
        ):
            # constants
            masks_f = pool_c1.tile([P, HG, 512], F32)
            nc.vector.memset(masks_f[:], 1.0)
            for o in range(HG):
                nc.gpsimd.affine_select(
                    out=masks_f[:, o, :], in_=masks_f[:, o, :],
                    compare_op=ALU.is_ge, fill=0.0, base=-P * o,
                    pattern=[[1, 512]], channel_multiplier=-1)
            masks = pool_c1.tile([P, HG, 512], BF16)
            nc.vector.tensor_copy(masks[:], masks_f[:])
            ones128 = pool_c1.tile([P, P], BF16)
            nc.vector.memset(ones128[:], 0.81)  # folds 1/KEEP^2; residual fixed in wo
            wo_sb = pool_wo.tile([P, DB, DHG], BF16)
            for db in range(DB):
                nc.sync.dma_start(wo_sb[:, db, :], _r(wo)[:, db, :])

            def head_tail(ao, h, pv, rsum):
                """ao[:,h,:] = pv / rsum (rowsum already replicated)."""
                recip = pool_recip.tile([P, 512], F32, tag="recip")
                nc.vector.reciprocal_approx_fast(recip[:], rsum[:])
                nc.vector.tensor_tensor(ao[:, h, :], pv[:], recip[:], ALU.mult)

            ag_outs = []

            def emit_D(t):
                bout_t = ag_outs[t]
                aof = pool_aof.tile([P, DB, 512], BF16, tag="aof")
                nc.sync.dma_start(aof[:], _r(bout_t))
                for si in range(4):
                    ssl = slice(si * P, (si + 1) * P)
                    ps = psD.tile([P, DHG], F32, tag="d")
                    for dbk in range(DB):
                        nc.tensor.matmul(ps[:], aof[:, dbk, ssl],
                                         wo_sb[:, dbk, :],
                                         start=(dbk == 0), stop=(dbk == DB - 1))
                    o_sb = pool_osb.tile([P, DHG], F32, tag="osb")
                    nc.scalar.activation(o_sb[:], ps[:], AF.Copy)
                    row0 = (t * 4 + si) * P
                    nc.sync.dma_start(out[row0:row0 + P, :], o_sb[:])

            for t in range(NT):
                tsl = slice(t * 512, (t + 1) * 512)
                ao = pool_ao.tile([P, HG, 512], BF16, tag="ao")
                nsj = 4 * (t + 1)
                prev = None
                for h in range(HG):
                    pv = psPV.tile([P, 512], F32, tag="pv")
                    rsum = psRS.tile([P, 512], F32, tag="rs")
                    for j in range(nsj):
                        jsl = slice(j * P, (j + 1) * P)
                        sc = psS.tile([P, 512], F32, tag="sc")
                        nc.tensor.matmul(sc[:], KT_sb[:, h, jsl],
                                         QT_sb[:, h, tsl], start=True, stop=True)
                        et = pool_exp.tile([P, 512], BF16, tag="et")
                        nc.scalar.activation(et[:], sc[:], AF.Exp,
                                             scale=EXP_SCALE)
                        o = j - 4 * t
                        if o >= 0:
                            nc.vector.tensor_tensor(et[:], et[:], masks[:, o, :],
                                                    ALU.mult)
                        nc.tensor.matmul(rsum[:], ones128[:], et[:],
                                         start=(j == 0), stop=(j == nsj - 1))
                        nc.tensor.matmul(pv[:], V_sb[:, j, h * P:(h + 1) * P],
                                         et[:], start=(j == 0),
                                         stop=(j == nsj - 1))
                        if j == 0 and prev is not None:
                            head_tail(ao, *prev)   # overlap prior head's tail
                            prev = None
                    prev = (h, pv, rsum)
                head_tail(ao, *prev)
                # AllGather this si-chunk across the 4-core group (bf16)
                bin_t = pool_dram.tile([DHG, 512], BF16, tag="bin")
                bout_t = pool_dram.tile([D, 512], BF16, tag="bout")
                nc.sync.dma_start(
                    bin_t.rearrange("(h p) s -> p h s", p=P), ao[:])
                nc.gpsimd.collective_compute(
                    "AllGather", ALU.bypass, ins=[bin_t[:].opt()],
                    outs=[bout_t[:].opt()], replica_groups=RGROUPS)
                ag_outs.append(bout_t)
                if t >= 2:
                    emit_D(t - 2)  # AG(t-2) is long done; spreads readback DMA
            emit_D(2)
            emit_D(3)


def _get_nc():
    if 'nc' not in _CACHE:
        _CACHE['nc'] = _build()
    return _CACHE['nc']


def _bf(a):
    return np.ascontiguousarray(a).astype(ml_dtypes.bfloat16)


def kernel(**inputs):
    x = np.asarray(inputs["x"], np.float32)
    g_Q = np.asarray(inputs["g_Q"], np.float32)
    g_K = np.asarray(inputs["g_K"], np.float32)
    g_V = np.asarray(inputs["g_V"], np.float32)
    qk_read = np.asarray(inputs["qk_read"], np.float32)
    qk_write = np.asarray(inputs["qk_write"], np.float32)
    v_read = np.asarray(inputs["v_read"], np.float32)
    v_write = np.asarray(inputs["v_write"], np.float32)
    W_O = np.asarray(inputs["W_O"], np.float32)

    nc = _get_nc()
    qk_readT = _bf(qk_read.T)
    v_readT = _bf(v_read.T)
    # the in-kernel rowsum scale is bf16(0.81); fold the exact residual
    # (and the intended 1/0.81 dropout scaling) into W_O host-side
    wo_fix = W_O * (KEEP2_BF / 0.81)
    in_maps = []
    for c in range(8):
        b, hg = divmod(c, 4)
        cs = slice(hg * DHG, (hg + 1) * DHG)
        in_maps.append({
            "xT": _bf(x[b].T),
            "gqT": _bf(g_Q[b].T),
            "gkT": _bf(g_K[b].T),
            "gvT": _bf(g_V[b].T),
            "qk_readT": qk_readT,
            "v_readT": v_readT,
            "qk_write_hg": _bf(qk_write[:, cs]),
            "v_write_hg": _bf(v_write[:, cs]),
            "wo_cols": _bf(wo_fix[:, cs]),
        })
    res = run_bass_kernel_spmd(nc, in_maps, core_ids=list(range(8)))
    _CACHE['last_results'] = res
    out = np.empty((B, S, D), np.float32)
    for c in range(8):
        b, hg = divmod(c, 4)
        out[b, :, hg * DHG:(hg + 1) * DHG] = res.results[c]["out"]
    return out


# revision 10
# speedup vs baseline: 1.2867x; 1.0100x over previous
"""Distributed Bass kernel for nn_AttentionCircuit (B=2,S=2048,D=2048,RANK=512,H=16).

Sharding: 8 cores = 2 batches x 4 head-groups (4 heads / 512 D-cols each).
All matmuls in bf16 (1 cycle/row on PE, half the DMA/SBUF of fp32).

Per-core dataflow (contraction always on the partition axis, no on-device
transposes; host pre-transposes x / gates):
  AB (fused, one pass over x^T, s-chunks of 512):
     t_qk^T = qk_read @ x^T ; gate -> Qg^T, Kg^T   (bf16)
     t_v^T  = v_read  @ x^T ; gate -> Vg^T
     Q^T/K^T = qk_write_hg.T @ {Q,K}g^T   (transposed [d',s], bf16 SBUF)
     V       = Vg^T.T @ v_write_hg        (natural [s,d'], bf16 SBUF)
  C: per 512-query chunk t, per head: scores^T = K^T.T Q^T -> exp (bf16, no
     max-sub; logits are tiny) -> causal mask (block-skip + diag masks) ->
     replicated rowsum via ones128-matmul -> PV matmul -> ao = pv * recip
     -> AllGather(group of 4, bf16) overlapped with next chunks
  D: out_cols = AO_full^T.T @ W_O[:,cols], chunks interleaved after AG(t)
     completes so the readback DMA spreads across phase C.
"""
import sys
import numpy as np
import ml_dtypes

sys.path.insert(0, '/opt/trn_rl_repo')

import concourse.bass as bass  # noqa: E402
from concourse import bacc  # noqa: E402
import concourse.mybir as mybir  # noqa: E402
import concourse.tile as tile  # noqa: E402
from concourse.bass_utils import run_bass_kernel_spmd  # noqa: E402

B, S, D = 2, 2048, 2048
RANK = 512
NH = 16
HG = 4              # head-groups == cores per batch
DHG = D // HG       # 512 cols per head-group (4 heads)
DH = D // NH        # 128 head dim
P = 128
DB = D // P         # 16 d-blocks
RB = RANK // P      # 4 rank-blocks
SB = S // P         # 16 s-blocks
NT = S // 512       # 4 si tiles of 512
SC = 512            # fused-A s-chunk width
NSC = S // SC       # 4

F32 = mybir.dt.float32
BF16 = mybir.dt.bfloat16
AF = mybir.ActivationFunctionType
ALU = mybir.AluOpType

EXP_SCALE = 1.0 / float(np.sqrt(DH))
KEEP2_BF = float(np.float32(ml_dtypes.bfloat16(0.81)))  # ones128 constant as hw sees it
RGROUPS = [[0, 1, 2, 3], [4, 5, 6, 7]]

_CACHE = {}


def _r(ap):
    """[ (o p), f ] DRAM tensor -> [p, o, f] partition-tiled view."""
    return ap.rearrange("(o p) f -> p o f", p=P)


def _build():
    nc = bacc.Bacc("TRN2", target_bir_lowering=False, debug=False,
                   enable_asserts=False, num_devices=8)
    xT = nc.dram_tensor("xT", [D, S], BF16, kind="ExternalInput").ap()
    gqT = nc.dram_tensor("gqT", [RANK, S], BF16, kind="ExternalInput").ap()
    gkT = nc.dram_tensor("gkT", [RANK, S], BF16, kind="ExternalInput").ap()
    gvT = nc.dram_tensor("gvT", [RANK, S], BF16, kind="ExternalInput").ap()
    qk_readT = nc.dram_tensor("qk_readT", [D, RANK], BF16, kind="ExternalInput").ap()
    v_readT = nc.dram_tensor("v_readT", [D, RANK], BF16, kind="ExternalInput").ap()
    qk_w = nc.dram_tensor("qk_write_hg", [RANK, DHG], BF16, kind="ExternalInput").ap()
    v_w = nc.dram_tensor("v_write_hg", [RANK, DHG], BF16, kind="ExternalInput").ap()
    wo = nc.dram_tensor("wo_cols", [D, DHG], BF16, kind="ExternalInput").ap()
    out = nc.dram_tensor("out", [S, DHG], F32, kind="ExternalOutput").ap()

    with tile.TileContext(nc) as tc:
        _body(tc, xT, gqT, gkT, gvT, qk_readT, v_readT, qk_w, v_w, wo, out)
    nc.compile()
    return nc


def _body(tc, xT, gqT, gkT, gvT, qk_readT, v_readT, qk_w, v_w, wo, out):
    nc = tc.nc
    import contextlib
    ctx = contextlib.ExitStack()
    with ctx:
        # ---- long-lived activation tensors
        pool_qk = ctx.enter_context(tc.tile_pool(name="qk", bufs=1))
        QT_sb = pool_qk.tile([P, HG, S], BF16)          # Q^T [d', s]
        KT_sb = pool_qk.tile([P, HG, S], BF16)
        V_sb = pool_qk.tile([P, SB, DHG], BF16)         # V natural [s, d']

        # ========== fused A+B per s-chunk: Q^T, K^T, V ==========
        with (
            tc.tile_pool(name="rd", bufs=1) as pool_rd,
            tc.tile_pool(name="wr", bufs=1) as pool_wr,
            tc.tile_pool(name="ax", bufs=2) as pool_x,
            tc.tile_pool(name="ag", bufs=2) as pool_g,
            tc.tile_pool(name="gch", bufs=2) as pool_gch,
            tc.tile_pool(name="psA", bufs=2, space="PSUM") as psA,
            tc.tile_pool(name="psB", bufs=2, space="PSUM") as psB,
        ):
            qr_sb = pool_rd.tile([P, DB, RANK], BF16)
            vr_sb = pool_rd.tile([P, DB, RANK], BF16)
            qw_sb = pool_wr.tile([P, RB, DHG], BF16)
            vw_sb = pool_wr.tile([P, RB, DHG], BF16)
            xt0 = pool_x.tile([P, DB, SC], BF16, tag="xt")
            # interleave the read-matrix / x-chunk-0 block DMAs so the first
            # A matmuls start after ~2 blocks land (subtile deps)
            for db in range(DB):
                nc.sync.dma_start(qr_sb[:, db, :], _r(qk_readT)[:, db, :])
                nc.sync.dma_start(xt0[:, db, :], _r(xT)[:, db, 0:SC])
            for db in range(DB):
                nc.sync.dma_start(vr_sb[:, db, :], _r(v_readT)[:, db, :])
            nc.sync.dma_start(qw_sb[:], _r(qk_w))
            nc.sync.dma_start(vw_sb[:], _r(v_w))
            for sc_i in range(NSC):
                sl = slice(sc_i * SC, (sc_i + 1) * SC)
                if sc_i == 0:
                    xt = xt0
                else:
                    xt = pool_x.tile([P, DB, SC], BF16, tag="xt")
                    nc.sync.dma_start(xt[:], _r(xT)[:, :, sl])
                gq = pool_g.tile([P, RB, SC], BF16, tag="gq")
                nc.sync.dma_start(gq[:], _r(gqT)[:, :, sl])
                gk = pool_g.tile([P, RB, SC], BF16, tag="gk")
                nc.sync.dma_start(gk[:], _r(gkT)[:, :, sl])
                gv = pool_g.tile([P, RB, SC], BF16, tag="gv")
                nc.sync.dma_start(gv[:], _r(gvT)[:, :, sl])
                qg = pool_gch.tile([P, RB, SC], BF16, tag="qg")
                kg = pool_gch.tile([P, RB, SC], BF16, tag="kg")
                vg = pool_gch.tile([P, RB, SC], BF16, tag="vg")
                for rb in range(RB):
                    ps = psA.tile([P, SC], F32, tag="tA")
                    for db in range(DB):
                        nc.tensor.matmul(ps[:], qr_sb[:, db, rb * P:(rb + 1) * P],
                                         xt[:, db, :], start=(db == 0),
                                         stop=(db == DB - 1))
                    nc.vector.tensor_tensor(qg[:, rb, :], ps[:], gq[:, rb, :],
                                            ALU.mult)
                    nc.vector.tensor_tensor(kg[:, rb, :], ps[:], gk[:, rb, :],
                                            ALU.mult)
                for rb in range(RB):
                    ps = psA.tile([P, SC], F32, tag="tA")
                    for db in range(DB):
                        nc.tensor.matmul(ps[:], vr_sb[:, db, rb * P:(rb + 1) * P],
                                         xt[:, db, :], start=(db == 0),
                                         stop=(db == DB - 1))
                    nc.vector.tensor_tensor(vg[:, rb, :], ps[:], gv[:, rb, :],
                                            ALU.mult)
                # B1: Q^T / K^T [d', s] per 128-col block of DHG
                for dbk in range(HG):
                    dsl = slice(dbk * P, (dbk + 1) * P)
                    psq = psB.tile([P, SC], F32, tag="qB")
                    for rb in range(RB):
                        nc.tensor.matmul(psq[:], qw_sb[:, rb, dsl], qg[:, rb, :],
                                         start=(rb == 0), stop=(rb == RB - 1))
                    nc.scalar.activation(QT_sb[:, dbk, sl], psq[:], AF.Copy)
                    psk = psB.tile([P, SC], F32, tag="kB")
                    for rb in range(RB):
                        nc.tensor.matmul(psk[:], qw_sb[:, rb, dsl], kg[:, rb, :],
                                         start=(rb == 0), stop=(rb == RB - 1))
                    nc.scalar.activation(KT_sb[:, dbk, sl], psk[:], AF.Copy)
                # B2: V natural [s, d']
                for sj in range(SC // P):
                    s_blk = sc_i * (SC // P) + sj
                    psv = psB.tile([P, DHG], F32, tag="vB")
                    for rb in range(RB):
                        nc.tensor.matmul(psv[:], vg[:, rb, sj * P:(sj + 1) * P],
                                         vw_sb[:, rb, :], start=(rb == 0),
                                         stop=(rb == RB - 1))
                    nc.scalar.activation(V_sb[:, s_blk, :], psv[:], AF.Copy)

        # ========== C + D: attention, AllGather, W_O ==========
        with (
            tc.tile_pool(name="csmall", bufs=1) as pool_c1,
            tc.tile_pool(name="exp", bufs=3) as pool_exp,
            tc.tile_pool(name="recip", bufs=2) as pool_recip,
            tc.tile_pool(name="ao", bufs=2) as pool_ao,
            tc.tile_pool(name="wo", bufs=1) as pool_wo,
            tc.tile_pool(name="aof", bufs=2) as pool_aof,
            tc.tile_pool(name="osb", bufs=2) as pool_osb,
            tc.tile_pool(name="dram0", bufs=1, space="DRAM") as pool_dram0,
            tc.tile_pool(name="dram1", bufs=1, space="DRAM") as pool_dram1,
            tc.tile_pool(name="dram2", bufs=1, space="DRAM") as pool_dram2,
            tc.tile_pool(name="dram3", bufs=1, space="DRAM") as pool_dram3,
            tc.tile_pool(name="psS", bufs=2, space="PSUM") as psS,
            tc.tile_pool(name="psPV", bufs=1, space="PSUM") as psPV,
            tc.tile_pool(name="psRS", bufs=1, space="PSUM") as psRS,
            tc.tile_pool(name="psD", bufs=2, space="PSUM") as psD,
        ):
            pool_drams = [pool_dram0, pool_dram1, pool_dram2, pool_dram3]
            # constants
            masks_f = pool_c1.tile([P, HG, 512], F32)
            nc.vector.memset(masks_f[:], 1.0)
            for o in range(HG):
                nc.gpsimd.affine_select(
                    out=masks_f[:, o, :], in_=masks_f[:, o, :],
                    compare_op=ALU.is_ge, fill=0.0, base=-P * o,
                    pattern=[[1, 512]], channel_multiplier=-1)
            masks = pool_c1.tile([P, HG, 512], BF16)
            nc.vector.tensor_copy(masks[:], masks_f[:])
            ones128 = pool_c1.tile([P, P], BF16)
            nc.vector.memset(ones128[:], 0.81)  # folds 1/KEEP^2; residual fixed in wo
            wo_sb = pool_wo.tile([P, DB, DHG], BF16)
            for db in range(DB):
                nc.sync.dma_start(wo_sb[:, db, :], _r(wo)[:, db, :])

            def head_tail(ao, h, pv, rsum):
                """ao[:,h,:] = pv / rsum (rowsum already replicated)."""
                recip = pool_recip.tile([P, 512], F32, tag="recip")
                nc.vector.reciprocal_approx_fast(recip[:], rsum[:])
                nc.vector.tensor_tensor(ao[:, h, :], pv[:], recip[:], ALU.mult)

            ag_outs = []

            def emit_D(t):
                bout_t = ag_outs[t]
                aof = pool_aof.tile([P, DB, 512], BF16, tag="aof")
                nc.sync.dma_start(aof[:], _r(bout_t))
                for si in range(4):
                    ssl = slice(si * P, (si + 1) * P)
                    ps = psD.tile([P, DHG], F32, tag="d")
                    for dbk in range(DB):
                        nc.tensor.matmul(ps[:], aof[:, dbk, ssl],
                                         wo_sb[:, dbk, :],
                                         start=(dbk == 0), stop=(dbk == DB - 1))
                    o_sb = pool_osb.tile([P, DHG], F32, tag="osb")
                    nc.scalar.activation(o_sb[:], ps[:], AF.Copy)
                    row0 = (t * 4 + si) * P
                    nc.sync.dma_start(out[row0:row0 + P, :], o_sb[:])

            for t in range(NT):
                tsl = slice(t * 512, (t + 1) * 512)
                ao = pool_ao.tile([P, HG, 512], BF16, tag="ao")
                npair = 2 * (t + 1)
                prev = None
                for h in range(HG):
                    pv = psPV.tile([P, 512], F32, tag="pv")
                    rsum = psRS.tile([P, 512], F32, tag="rs")
                    for p in range(npair):
                        # paired key blocks j=2p,2p+1: 2 score matmuls into one
                        # 2-bank PSUM tile, a single exp over 1024 cols keeps
                        # ScalarE off the per-block critical path
                        scp = psS.tile([P, 2, 512], F32, tag="sc")
                        for i in range(2):
                            j = 2 * p + i
                            nc.tensor.matmul(scp[:, i, :],
                                             KT_sb[:, h, j * P:(j + 1) * P],
                                             QT_sb[:, h, tsl],
                                             start=True, stop=True)
                        etp = pool_exp.tile([P, 2, 512], BF16, tag="et")
                        nc.scalar.activation(etp[:], scp[:], AF.Exp,
                                             scale=EXP_SCALE)
                        dp = p - 2 * t
                        if dp >= 0:
                            nc.vector.tensor_tensor(
                                etp[:], etp[:],
                                masks[:, 2 * dp:2 * dp + 2, :], ALU.mult)
                        for i in range(2):
                            j = 2 * p + i
                            nc.tensor.matmul(rsum[:], ones128[:], etp[:, i, :],
                                             start=(j == 0),
                                             stop=(j == 2 * npair - 1))
                            nc.tensor.matmul(pv[:],
                                             V_sb[:, j, h * P:(h + 1) * P],
                                             etp[:, i, :], start=(j == 0),
                                             stop=(j == 2 * npair - 1))
                        if p == 0 and prev is not None:
                            head_tail(ao, *prev)   # overlap prior head's tail
                            prev = None
                    prev = (h, pv, rsum)
                head_tail(ao, *prev)
                # AllGather this si-chunk across the 4-core group (bf16).
                # bin/bout live in per-chunk DRAM pools so chunk t+1's staging
                # DMA is not serialized behind collective t.
                bin_t = pool_drams[t].tile([DHG, 512], BF16, tag="bin")
                bout_t = pool_drams[t].tile([D, 512], BF16, tag="bout")
                nc.sync.dma_start(
                    bin_t.rearrange("(h p) s -> p h s", p=P), ao[:])
                nc.gpsimd.collective_compute(
                    "AllGather", ALU.bypass, ins=[bin_t[:].opt()],
                    outs=[bout_t[:].opt()], replica_groups=RGROUPS)
                ag_outs.append(bout_t)
                if t >= 2:
                    emit_D(t - 2)  # AG(t-2) is long done; spreads readback DMA
            emit_D(2)
            emit_D(3)


def _get_nc():
    if 'nc' not in _CACHE:
        _CACHE['nc'] = _build()
    return _CACHE['nc']


def _bf(a):
    return np.ascontiguousarray(a).astype(ml_dtypes.bfloat16)


def kernel(**inputs):
    x = np.asarray(inputs["x"], np.float32)
    g_Q = np.asarray(inputs["g_Q"], np.float32)
    g_K = np.asarray(inputs["g_K"], np.float32)
    g_V = np.asarray(inputs["g_V"], np.float32)
    qk_read = np.asarray(inputs["qk_read"], np.float32)
    qk_write = np.asarray(inputs["qk_write"], np.float32)
    v_read = np.asarray(inputs["v_read"], np.float32)
    v_write = np.asarray(inputs["v_write"], np.float32)
    W_O = np.asarray(inputs["W_O"], np.float32)

    nc = _get_nc()
    qk_readT = _bf(qk_read.T)
    v_readT = _bf(v_read.T)
    # the in-kernel rowsum scale is bf16(0.81); fold the exact residual
    # (and the intended 1/0.81 dropout scaling) into W_O host-side
    wo_fix = W_O * (KEEP2_BF / 0.81)
    in_maps = []
    for c in range(8):
        b, hg = divmod(c, 4)
        cs = slice(hg * DHG, (hg + 1) * DHG)
        in_maps.append({
            "xT": _bf(x[b].T),
            "gqT": _bf(g_Q[b].T),
            "gkT": _bf(g_K[b].T),
            "gvT": _bf(g_V[b].T),
            "qk_readT": qk_readT,
            "v_readT": v_readT,
            "qk_write_hg": _bf(qk_write[:, cs]),
            "v_write_hg": _bf(v_write[:, cs]),
            "wo_cols": _bf(wo_fix[:, cs]),
        })
    res = run_bass_kernel_spmd(nc, in_maps, core_ids=list(range(8)))
    _CACHE['last_results'] = res
    out = np.empty((B, S, D), np.float32)
    for c in range(8):
        b, hg = divmod(c, 4)
        out[b, :, hg * DHG:(hg + 1) * DHG] = res.results[c]["out"]
    return out


# revision 12
# speedup vs baseline: 1.3149x; 1.0219x over previous
"""Distributed Bass kernel for nn_AttentionCircuit (B=2,S=2048,D=2048,RANK=512,H=16).

Sharding: 8 cores = 2 batches x 4 head-groups (4 heads / 512 D-cols each).
All matmuls in bf16 (1 cycle/row on PE, half the DMA/SBUF of fp32).

Per-core dataflow (contraction always on the partition axis, no on-device
transposes; host pre-transposes x / gates):
  AB (fused, one pass over x^T, s-chunks of 512):
     t_qk^T = qk_read @ x^T ; gate -> Qg^T, Kg^T   (bf16)
     t_v^T  = v_read  @ x^T ; gate -> Vg^T
     Q^T/K^T = qk_write_hg.T @ {Q,K}g^T   (transposed [d',s], bf16 SBUF)
     V       = Vg^T.T @ v_write_hg        (natural [s,d'], bf16 SBUF)
  C: per 512-query chunk t, per head: scores^T = K^T.T Q^T -> exp (bf16, no
     max-sub; logits are tiny) -> causal mask (block-skip + diag masks) ->
     replicated rowsum via ones128-matmul -> PV matmul -> ao = pv * recip
     -> AllGather(group of 4, bf16) overlapped with next chunks
  D: out_cols = AO_full^T.T @ W_O[:,cols], chunks interleaved after AG(t)
     completes so the readback DMA spreads across phase C.
"""
import sys
import numpy as np
import ml_dtypes

sys.path.insert(0, '/opt/trn_rl_repo')

import concourse.bass as bass  # noqa: E402
from concourse import bacc  # noqa: E402
import concourse.mybir as mybir  # noqa: E402
import concourse.tile as tile  # noqa: E402
from concourse.bass_utils import run_bass_kernel_spmd  # noqa: E402

B, S, D = 2, 2048, 2048
RANK = 512
NH = 16
HG = 4              # head-groups == cores per batch
DHG = D // HG       # 512 cols per head-group (4 heads)
DH = D // NH        # 128 head dim
P = 128
DB = D // P         # 16 d-blocks
RB = RANK // P      # 4 rank-blocks
SB = S // P         # 16 s-blocks
NT = S // 512       # 4 si tiles of 512
SC = 512            # fused-A s-chunk width
NSC = S // SC       # 4

F32 = mybir.dt.float32
BF16 = mybir.dt.bfloat16
AF = mybir.ActivationFunctionType
ALU = mybir.AluOpType

EXP_SCALE = 1.0 / float(np.sqrt(DH))
KEEP2_BF = float(np.float32(ml_dtypes.bfloat16(0.81)))  # ones128 constant as hw sees it
RGROUPS = [[0, 1, 2, 3], [4, 5, 6, 7]]

_CACHE = {}


def _r(ap):
    """[ (o p), f ] DRAM tensor -> [p, o, f] partition-tiled view."""
    return ap.rearrange("(o p) f -> p o f", p=P)


def _build():
    nc = bacc.Bacc("TRN2", target_bir_lowering=False, debug=False,
                   enable_asserts=False, num_devices=8)
    xT = nc.dram_tensor("xT", [D, S], BF16, kind="ExternalInput").ap()
    gqT = nc.dram_tensor("gqT", [RANK, S], BF16, kind="ExternalInput").ap()
    gkT = nc.dram_tensor("gkT", [RANK, S], BF16, kind="ExternalInput").ap()
    gvT = nc.dram_tensor("gvT", [RANK, S], BF16, kind="ExternalInput").ap()
    qk_readT = nc.dram_tensor("qk_readT", [D, RANK], BF16, kind="ExternalInput").ap()
    v_readT = nc.dram_tensor("v_readT", [D, RANK], BF16, kind="ExternalInput").ap()
    qk_w = nc.dram_tensor("qk_write_hg", [RANK, DHG], BF16, kind="ExternalInput").ap()
    v_w = nc.dram_tensor("v_write_hg", [RANK, DHG], BF16, kind="ExternalInput").ap()
    wo = nc.dram_tensor("wo_cols", [D, DHG], BF16, kind="ExternalInput").ap()
    out = nc.dram_tensor("out", [S, DHG], F32, kind="ExternalOutput").ap()

    with tile.TileContext(nc) as tc:
        _body(tc, xT, gqT, gkT, gvT, qk_readT, v_readT, qk_w, v_w, wo, out)
    nc.compile()
    return nc


def _body(tc, xT, gqT, gkT, gvT, qk_readT, v_readT, qk_w, v_w, wo, out):
    nc = tc.nc
    import contextlib
    ctx = contextlib.ExitStack()
    with ctx:
        # ---- long-lived activation tensors
        pool_qk = ctx.enter_context(tc.tile_pool(name="qk", bufs=1))
        QT_sb = pool_qk.tile([P, HG, S], BF16)          # Q^T [d', s]
        KT_sb = pool_qk.tile([P, HG, S], BF16)
        V_sb = pool_qk.tile([P, SB, DHG], BF16)         # V natural [s, d']

        # ========== fused A+B per s-chunk: Q^T, K^T, V ==========
        with (
            tc.tile_pool(name="rd", bufs=1) as pool_rd,
            tc.tile_pool(name="wr", bufs=1) as pool_wr,
            tc.tile_pool(name="ax", bufs=2) as pool_x,
            tc.tile_pool(name="ag", bufs=2) as pool_g,
            tc.tile_pool(name="gch", bufs=2) as pool_gch,
            tc.tile_pool(name="psA", bufs=2, space="PSUM") as psA,
            tc.tile_pool(name="psB", bufs=2, space="PSUM") as psB,
        ):
            qr_sb = pool_rd.tile([P, DB, RANK], BF16)
            vr_sb = pool_rd.tile([P, DB, RANK], BF16)
            qw_sb = pool_wr.tile([P, RB, DHG], BF16)
            vw_sb = pool_wr.tile([P, RB, DHG], BF16)
            xt0 = pool_x.tile([P, DB, SC], BF16, tag="xt")
            gq0 = pool_g.tile([P, RB, SC], BF16, tag="gq")
            gk0 = pool_g.tile([P, RB, SC], BF16, tag="gk")
            gv0 = pool_g.tile([P, RB, SC], BF16, tag="gv")
            # interleave the read-matrix / x-chunk-0 block DMAs so the first
            # A matmuls start after ~2 blocks land (subtile deps); chunk-0
            # gates right behind so rb=0's gating isn't starved
            for db in range(DB):
                nc.sync.dma_start(qr_sb[:, db, :], _r(qk_readT)[:, db, :])
                nc.sync.dma_start(xt0[:, db, :], _r(xT)[:, db, 0:SC])
                if db == 1:
                    nc.sync.dma_start(gq0[:], _r(gqT)[:, :, 0:SC])
                    nc.sync.dma_start(gk0[:], _r(gkT)[:, :, 0:SC])
                    nc.sync.dma_start(gv0[:], _r(gvT)[:, :, 0:SC])
            for db in range(DB):
                nc.sync.dma_start(vr_sb[:, db, :], _r(v_readT)[:, db, :])
            nc.sync.dma_start(qw_sb[:], _r(qk_w))
            nc.sync.dma_start(vw_sb[:], _r(v_w))
            for sc_i in range(NSC):
                sl = slice(sc_i * SC, (sc_i + 1) * SC)
                if sc_i == 0:
                    xt, gq, gk, gv = xt0, gq0, gk0, gv0
                else:
                    xt = pool_x.tile([P, DB, SC], BF16, tag="xt")
                    nc.sync.dma_start(xt[:], _r(xT)[:, :, sl])
                    gq = pool_g.tile([P, RB, SC], BF16, tag="gq")
                    nc.sync.dma_start(gq[:], _r(gqT)[:, :, sl])
                    gk = pool_g.tile([P, RB, SC], BF16, tag="gk")
                    nc.sync.dma_start(gk[:], _r(gkT)[:, :, sl])
                    gv = pool_g.tile([P, RB, SC], BF16, tag="gv")
                    nc.sync.dma_start(gv[:], _r(gvT)[:, :, sl])
                qg = pool_gch.tile([P, RB, SC], BF16, tag="qg")
                kg = pool_gch.tile([P, RB, SC], BF16, tag="kg")
                vg = pool_gch.tile([P, RB, SC], BF16, tag="vg")
                for rb in range(RB):
                    ps = psA.tile([P, SC], F32, tag="tA")
                    for db in range(DB):
                        nc.tensor.matmul(ps[:], qr_sb[:, db, rb * P:(rb + 1) * P],
                                         xt[:, db, :], start=(db == 0),
                                         stop=(db == DB - 1))
                    nc.vector.tensor_tensor(qg[:, rb, :], ps[:], gq[:, rb, :],
                                            ALU.mult)
                    nc.vector.tensor_tensor(kg[:, rb, :], ps[:], gk[:, rb, :],
                                            ALU.mult)
                for rb in range(RB):
                    ps = psA.tile([P, SC], F32, tag="tA")
                    for db in range(DB):
                        nc.tensor.matmul(ps[:], vr_sb[:, db, rb * P:(rb + 1) * P],
                                         xt[:, db, :], start=(db == 0),
                                         stop=(db == DB - 1))
                    nc.vector.tensor_tensor(vg[:, rb, :], ps[:], gv[:, rb, :],
                                            ALU.mult)
                # B1: Q^T / K^T [d', s] per 128-col block of DHG
                for dbk in range(HG):
                    dsl = slice(dbk * P, (dbk + 1) * P)
                    psq = psB.tile([P, SC], F32, tag="qB")
                    for rb in range(RB):
                        nc.tensor.matmul(psq[:], qw_sb[:, rb, dsl], qg[:, rb, :],
                                         start=(rb == 0), stop=(rb == RB - 1))
                    nc.scalar.activation(QT_sb[:, dbk, sl], psq[:], AF.Copy)
                    psk = psB.tile([P, SC], F32, tag="kB")
                    for rb in range(RB):
                        nc.tensor.matmul(psk[:], qw_sb[:, rb, dsl], kg[:, rb, :],
                                         start=(rb == 0), stop=(rb == RB - 1))
                    nc.scalar.activation(KT_sb[:, dbk, sl], psk[:], AF.Copy)
                # B2: V natural [s, d']
                for sj in range(SC // P):
                    s_blk = sc_i * (SC // P) + sj
                    psv = psB.tile([P, DHG], F32, tag="vB")
                    for rb in range(RB):
                        nc.tensor.matmul(psv[:], vg[:, rb, sj * P:(sj + 1) * P],
                                         vw_sb[:, rb, :], start=(rb == 0),
                                         stop=(rb == RB - 1))
                    nc.scalar.activation(V_sb[:, s_blk, :], psv[:], AF.Copy)

        # ========== C + D: attention, AllGather, W_O ==========
        with (
            tc.tile_pool(name="csmall", bufs=1) as pool_c1,
            tc.tile_pool(name="exp", bufs=3) as pool_exp,
            tc.tile_pool(name="recip", bufs=2) as pool_recip,
            tc.tile_pool(name="ao", bufs=2) as pool_ao,
            tc.tile_pool(name="wo", bufs=1) as pool_wo,
            tc.tile_pool(name="aof", bufs=2) as pool_aof,
            tc.tile_pool(name="osb", bufs=2) as pool_osb,
            tc.tile_pool(name="dram0", bufs=1, space="DRAM") as pool_dram0,
            tc.tile_pool(name="dram1", bufs=1, space="DRAM") as pool_dram1,
            tc.tile_pool(name="dram2", bufs=1, space="DRAM") as pool_dram2,
            tc.tile_pool(name="dram3", bufs=1, space="DRAM") as pool_dram3,
            tc.tile_pool(name="psS", bufs=2, space="PSUM") as psS,
            tc.tile_pool(name="psPV", bufs=1, space="PSUM") as psPV,
            tc.tile_pool(name="psRS", bufs=1, space="PSUM") as psRS,
            tc.tile_pool(name="psD", bufs=2, space="PSUM") as psD,
        ):
            pool_drams = [pool_dram0, pool_dram1, pool_dram2, pool_dram3]
            # constants
            masks_f = pool_c1.tile([P, HG, 512], F32)
            nc.vector.memset(masks_f[:], 1.0)
            for o in range(HG):
                nc.gpsimd.affine_select(
                    out=masks_f[:, o, :], in_=masks_f[:, o, :],
                    compare_op=ALU.is_ge, fill=0.0, base=-P * o,
                    pattern=[[1, 512]], channel_multiplier=-1)
            masks = pool_c1.tile([P, HG, 512], BF16)
            nc.vector.tensor_copy(masks[:], masks_f[:])
            ones128 = pool_c1.tile([P, P], BF16)
            nc.vector.memset(ones128[:], 0.81)  # folds 1/KEEP^2; residual fixed in wo
            wo_sb = pool_wo.tile([P, DB, DHG], BF16)
            for db in range(DB):
                nc.sync.dma_start(wo_sb[:, db, :], _r(wo)[:, db, :])

            def head_tail(ao, h, pv, rsum):
                """ao[:,h,:] = pv / rsum (rowsum already replicated)."""
                recip = pool_recip.tile([P, 512], F32, tag="recip")
                nc.vector.reciprocal_approx_fast(recip[:], rsum[:])
                nc.vector.tensor_tensor(ao[:, h, :], pv[:], recip[:], ALU.mult)

            ag_outs = []

            def emit_D(t):
                bout_t = ag_outs[t]
                aof = pool_aof.tile([P, DB, 512], BF16, tag="aof")
                nc.sync.dma_start(aof[:], _r(bout_t))
                for si in range(4):
                    ssl = slice(si * P, (si + 1) * P)
                    ps = psD.tile([P, DHG], F32, tag="d")
                    for dbk in range(DB):
                        nc.tensor.matmul(ps[:], aof[:, dbk, ssl],
                                         wo_sb[:, dbk, :],
                                         start=(dbk == 0), stop=(dbk == DB - 1))
                    o_sb = pool_osb.tile([P, DHG], F32, tag="osb")
                    nc.scalar.activation(o_sb[:], ps[:], AF.Copy)
                    row0 = (t * 4 + si) * P
                    nc.sync.dma_start(out[row0:row0 + P, :], o_sb[:])

            for t in range(NT):
                tsl = slice(t * 512, (t + 1) * 512)
                ao = pool_ao.tile([P, HG, 512], BF16, tag="ao")
                npair = 2 * (t + 1)
                prev = None
                for h in range(HG):
                    pv = psPV.tile([P, 512], F32, tag="pv")
                    rsum = psRS.tile([P, 512], F32, tag="rs")
                    for p in range(npair):
                        # paired key blocks j=2p,2p+1: 2 score matmuls into one
                        # 2-bank PSUM tile, a single exp over 1024 cols keeps
                        # ScalarE off the per-block critical path
                        scp = psS.tile([P, 2, 512], F32, tag="sc")
                        for i in range(2):
                            j = 2 * p + i
                            nc.tensor.matmul(scp[:, i, :],
                                             KT_sb[:, h, j * P:(j + 1) * P],
                                             QT_sb[:, h, tsl],
                                             start=True, stop=True)
                        etp = pool_exp.tile([P, 2, 512], BF16, tag="et")
                        nc.scalar.activation(etp[:], scp[:], AF.Exp,
                                             scale=EXP_SCALE)
                        dp = p - 2 * t
                        if dp >= 0:
                            nc.vector.tensor_tensor(
                                etp[:], etp[:],
                                masks[:, 2 * dp:2 * dp + 2, :], ALU.mult)
                        for i in range(2):
                            j = 2 * p + i
                            nc.tensor.matmul(rsum[:], ones128[:], etp[:, i, :],
                                             start=(j == 0),
                                             stop=(j == 2 * npair - 1))
                            nc.tensor.matmul(pv[:],
                                             V_sb[:, j, h * P:(h + 1) * P],
                                             etp[:, i, :], start=(j == 0),
                                             stop=(j == 2 * npair - 1))
                        if p == 0 and prev is not None:
                            head_tail(ao, *prev)   # overlap prior head's tail
                            prev = None
                    prev = (h, pv, rsum)
                head_tail(ao, *prev)
                # AllGather this si-chunk across the 4-core group (bf16).
                # bin/bout live in per-chunk DRAM pools so chunk t+1's staging
                # DMA is not serialized behind collective t.
                bin_t = pool_drams[t].tile([DHG, 512], BF16, tag="bin")
                bout_t = pool_drams[t].tile([D, 512], BF16, tag="bout")
                nc.sync.dma_start(
                    bin_t.rearrange("(h p) s -> p h s", p=P), ao[:])
                nc.gpsimd.collective_compute(
                    "AllGather", ALU.bypass, ins=[bin_t[:].opt()],
                    outs=[bout_t[:].opt()], replica_groups=RGROUPS)
                ag_outs.append(bout_t)
            # All D after C: aof(0..2) are ready well before the PE sequencer
            # reaches them (a not-yet-satisfied wait in the PE stream stalls
            # dispatch far ahead of the array), and D(0..2) buffers enough PE
            # work to cover AG(3)'s latency before D(3).
            for t in range(NT):
                emit_D(t)


def _get_nc():
    if 'nc' not in _CACHE:
        _CACHE['nc'] = _build()
    return _CACHE['nc']


def _bf(a):
    return np.ascontiguousarray(a).astype(ml_dtypes.bfloat16)


def kernel(**inputs):
    x = np.asarray(inputs["x"], np.float32)
    g_Q = np.asarray(inputs["g_Q"], np.float32)
    g_K = np.asarray(inputs["g_K"], np.float32)
    g_V = np.asarray(inputs["g_V"], np.float32)
    qk_read = np.asarray(inputs["qk_read"], np.float32)
    qk_write = np.asarray(inputs["qk_write"], np.float32)
    v_read = np.asarray(inputs["v_read"], np.float32)
    v_write = np.asarray(inputs["v_write"], np.float32)
    W_O = np.asarray(inputs["W_O"], np.float32)

    nc = _get_nc()
    qk_readT = _bf(qk_read.T)
    v_readT = _bf(v_read.T)
    # the in-kernel rowsum scale is bf16(0.81); fold the exact residual
    # (and the intended 1/0.81 dropout scaling) into W_O host-side
    wo_fix = W_O * (KEEP2_BF / 0.81)
    in_maps = []
    for c in range(8):
        b, hg = divmod(c, 4)
        cs = slice(hg * DHG, (hg + 1) * DHG)
        in_maps.append({
            "xT": _bf(x[b].T),
            "gqT": _bf(g_Q[b].T),
            "gkT": _bf(g_K[b].T),
            "gvT": _bf(g_V[b].T),
            "qk_readT": qk_readT,
            "v_readT": v_readT,
            "qk_write_hg": _bf(qk_write[:, cs]),
            "v_write_hg": _bf(v_write[:, cs]),
            "wo_cols": _bf(wo_fix[:, cs]),
        })
    res = run_bass_kernel_spmd(nc, in_maps, core_ids=list(range(8)))
    _CACHE['last_results'] = res
    out = np.empty((B, S, D), np.float32)
    for c in range(8):
        b, hg = divmod(c, 4)
        out[b, :, hg * DHG:(hg + 1) * DHG] = res.results[c]["out"]
    return out
